# revision 1
# baseline (speedup 1.0000x reference)
"""Block-diagonal linear (grouped GEMM) on 8 TRN2 NeuronCores.

out[b, g*512+n] = sum_k x[b, g*512+k] * blocks[g, k, n]

Sharding: group-parallel — core g computes block g's GEMM. The host hands
each core xT = x[:, g*512:(g+1)*512].T ([512, 8192], feature-major) and
receives outT ([512, 8192]); the transposes happen on the host so the
device needs no PE transposes and every DMA stream reads/writes long
contiguous runs per partition.

Per-core kernel: out.T = W.T @ x.T as 64 PSUM accumulation groups:
psum[n-tile 128, m 512] += W[k-tile, n-tile].T @ xT[k-tile, m-chunk],
with all matmul operands rounded to float32r (full PE rate at N=512,
~1.5e-4 max rel err vs fp32).
"""
import numpy as np

import concourse.bacc as bacc
import concourse.tile as tile
from concourse import mybir
from concourse.bass_utils import run_bass_kernel_spmd

TOKENS = 8192
G = 8
M = 512  # per-block in-features
N = 512  # per-block out-features
P = 128
KT = M // P  # 4 contraction tiles
NT = N // P  # 4 output feature tiles
SUB = 512    # tokens per PSUM group (moving-dim max for 4-byte dtypes)
F32 = mybir.dt.float32
F32R = mybir.dt.float32r

# token-chunk schedule: small head/tail for pipeline ramp, 2048 steady
CHUNKS = [512, 512, 1024, 2048, 2048, 1024, 512, 512]
assert sum(CHUNKS) == TOKENS
CMAX = max(CHUNKS)

_CACHE: dict = {}


def _body(tc, nc, xT, w, outT):
    with (
        tc.tile_pool(name="wp", bufs=1) as wp,
        tc.tile_pool(name="xin", bufs=12) as xin,
        tc.tile_pool(name="outp", bufs=2) as outp,
        tc.tile_pool(name="pso", bufs=8, space="PSUM") as pso,
    ):
        # weights [512, 512] -> [128, kt, 512] fp32, rounded once to f32r
        w_f = wp.tile([P, KT, N], F32, tag="wf")
        w_r = wp.tile([P, KT, N], F32R, tag="wr")
        w_v = w.rearrange("(j p) n -> j p n", p=P)

        m0 = 0
        for ci, c in enumerate(CHUNKS):
            # load + round the 4 k-tiles of this token chunk, striped across
            # the two HWDGE rings (sync=SP and scalar=ACT)
            xs = []
            for j in range(KT):
                x_t = xin.tile([P, CMAX], F32R, tag="x")
                eng = nc.sync if j % 2 == 0 else nc.scalar
                eng.dma_start(
                    x_t[:, :c], xT[j * P:(j + 1) * P, m0:m0 + c].bitcast(F32R)
                )
                xs.append(x_t)
            if ci == 0:
                # W rides both rings right behind the first chunk
                for j in range(KT):
                    eng = nc.sync if j % 2 == 0 else nc.scalar
                    eng.dma_start(w_f[:, j, :], w_v[j])
                    nc.vector.tensor_copy(w_r[:, j, :], w_f[:, j, :])

            ots = [outp.tile([P, CMAX], F32, tag=f"o{nt}", name=f"ot{nt}") for nt in range(NT)]
            for s0 in range(0, c, SUB):
                sw = min(SUB, c - s0)
                for nt in range(NT):
                    ps_o = pso.tile([P, SUB], F32, tag="pso")
                    for j in range(KT):
                        nc.tensor.matmul(
                            ps_o[:, :sw],
                            w_r[:, j, nt * P:(nt + 1) * P],
                            xs[j][:, s0:s0 + sw],
                            start=(j == 0),
                            stop=(j == KT - 1),
                        )
                    nc.vector.tensor_copy(ots[nt][:, s0:s0 + sw], ps_o[:, :sw])
            # flush the chunk: one DMA per n-tile on the SWDGE ring; the last
            # chunks ride the HWDGE rings (input traffic is done by then)
            for nt in range(NT):
                if ci >= len(CHUNKS) - 3:
                    eng = nc.sync if nt % 2 == 0 else nc.scalar
                else:
                    eng = nc.gpsimd
                eng.dma_start(outT[nt * P:(nt + 1) * P, m0:m0 + c], ots[nt][:, :c])
            m0 += c


def _build():
    nc = bacc.Bacc("TRN2", target_bir_lowering=False, debug=False, num_devices=G)
    xT = nc.dram_tensor("xT", [M, TOKENS], F32, kind="ExternalInput").ap()
    w = nc.dram_tensor("w", [M, N], F32, kind="ExternalInput").ap()
    outT = nc.dram_tensor("outT", [N, TOKENS], F32, kind="ExternalOutput").ap()
    with tile.TileContext(nc) as tc:
        _body(tc, nc, xT, w, outT)
    nc.compile()
    return nc


def _run(in_maps, **kwargs):
    if "nc" not in _CACHE:
        _CACHE["nc"] = _build()
    return run_bass_kernel_spmd(_CACHE["nc"], in_maps, list(range(G)), **kwargs)


def _in_maps(x, blocks):
    return [
        {
            "xT": np.ascontiguousarray(x[:, g * M:(g + 1) * M].T, dtype=np.float32),
            "w": np.ascontiguousarray(blocks[g], dtype=np.float32),
        }
        for g in range(G)
    ]


def kernel(x, blocks):
    x = np.asarray(x)
    blocks = np.asarray(blocks)
    res = _run(_in_maps(x, blocks))
    return np.concatenate(
        [res.results[g]["outT"].T for g in range(G)], axis=1
    ).astype(np.float32, copy=False)



# revision 2
# speedup vs baseline: 1.1893x; 1.1893x over previous
"""Block-diagonal linear (grouped GEMM) on 8 TRN2 NeuronCores.

out[b, g*512+n] = sum_k x[b, g*512+k] * blocks[g, k, n]

Sharding: group-parallel — core g computes block g's GEMM. The host hands
each core xT = x[:, g*512:(g+1)*512].T ([512, 8192], feature-major) in
bf16 and receives outT ([512, 8192]) in bf16; transposes and dtype
conversion happen on the host so the device needs no PE transposes and
every DMA stream reads/writes long contiguous runs per partition.

bf16 halves HBM traffic vs fp32 (the fp32 version was DMA-bound at
~34.6MB/core ≈ 96µs; bf16 is ~17.3MB ≈ 48µs) while the PE runs bf16 at
the same 1 cycle/row as f32r, so the kernel becomes compute-bound at
~55µs. Accumulation stays fp32 in PSUM; end-to-end max rel err vs the
fp32 reference is ~4e-3 (gate 2e-2).

Per-core kernel: out.T = W.T @ x.T as 64 PSUM accumulation groups:
psum[n-tile 128, m 512] += W[k-tile, n-tile].T @ xT[k-tile, m-chunk].
"""
import numpy as np
import ml_dtypes

import concourse.bacc as bacc
import concourse.tile as tile
from concourse import mybir
from concourse.bass_utils import run_bass_kernel_spmd

TOKENS = 8192
G = 8
M = 512  # per-block in-features
N = 512  # per-block out-features
P = 128
KT = M // P  # 4 contraction tiles
NT = N // P  # 4 output feature tiles
SUB = 512    # tokens per PSUM group (one fp32 PSUM bank)
F32 = mybir.dt.float32
BF16 = mybir.dt.bfloat16
NPBF16 = ml_dtypes.bfloat16

# token-chunk schedule: small head/tail for pipeline ramp, 2048 steady
CHUNKS = [512, 512, 1024, 2048, 2048, 1024, 512, 512]
assert sum(CHUNKS) == TOKENS
CMAX = max(CHUNKS)

_CACHE: dict = {}


def _body(tc, nc, xT, w, outT):
    with (
        tc.tile_pool(name="wp", bufs=1) as wp,
        tc.tile_pool(name="xin", bufs=12) as xin,
        tc.tile_pool(name="outp", bufs=2) as outp,
        tc.tile_pool(name="pso", bufs=8, space="PSUM") as pso,
    ):
        # weights [512, 512] bf16 -> [128, kt, 512]
        w_r = wp.tile([P, KT, N], BF16, tag="wr")
        w_v = w.rearrange("(j p) n -> j p n", p=P)

        m0 = 0
        for ci, c in enumerate(CHUNKS):
            # load the 4 k-tiles of this token chunk, striped across
            # the two HWDGE rings (sync=SP and scalar=ACT)
            xs = []
            for j in range(KT):
                x_t = xin.tile([P, CMAX], BF16, tag="x")
                eng = nc.sync if j % 2 == 0 else nc.scalar
                eng.dma_start(x_t[:, :c], xT[j * P:(j + 1) * P, m0:m0 + c])
                xs.append(x_t)
            if ci == 0:
                # W rides both rings right behind the first chunk
                for j in range(KT):
                    eng = nc.sync if j % 2 == 0 else nc.scalar
                    eng.dma_start(w_r[:, j, :], w_v[j])

            ots = [outp.tile([P, CMAX], BF16, tag=f"o{nt}", name=f"ot{nt}") for nt in range(NT)]
            for s0 in range(0, c, SUB):
                sw = min(SUB, c - s0)
                for nt in range(NT):
                    ps_o = pso.tile([P, SUB], F32, tag="pso")
                    for j in range(KT):
                        nc.tensor.matmul(
                            ps_o[:, :sw],
                            w_r[:, j, nt * P:(nt + 1) * P],
                            xs[j][:, s0:s0 + sw],
                            start=(j == 0),
                            stop=(j == KT - 1),
                        )
                    nc.vector.tensor_copy(ots[nt][:, s0:s0 + sw], ps_o[:, :sw])
            # flush the chunk: one DMA per n-tile on the SWDGE ring; the last
            # chunks ride the HWDGE rings (input traffic is done by then)
            for nt in range(NT):
                if ci >= len(CHUNKS) - 3:
                    eng = nc.sync if nt % 2 == 0 else nc.scalar
                else:
                    eng = nc.gpsimd
                eng.dma_start(outT[nt * P:(nt + 1) * P, m0:m0 + c], ots[nt][:, :c])
            m0 += c


def _build():
    nc = bacc.Bacc("TRN2", target_bir_lowering=False, debug=False, num_devices=G)
    xT = nc.dram_tensor("xT", [M, TOKENS], BF16, kind="ExternalInput").ap()
    w = nc.dram_tensor("w", [M, N], BF16, kind="ExternalInput").ap()
    outT = nc.dram_tensor("outT", [N, TOKENS], BF16, kind="ExternalOutput").ap()
    with tile.TileContext(nc) as tc:
        _body(tc, nc, xT, w, outT)
    nc.compile()
    return nc


def _run(in_maps, **kwargs):
    if "nc" not in _CACHE:
        _CACHE["nc"] = _build()
    return run_bass_kernel_spmd(_CACHE["nc"], in_maps, list(range(G)), **kwargs)


def _in_maps(x, blocks):
    return [
        {
            "xT": np.ascontiguousarray(x[:, g * M:(g + 1) * M].T).astype(NPBF16),
            "w": np.ascontiguousarray(blocks[g]).astype(NPBF16),
        }
        for g in range(G)
    ]


def kernel(x, blocks):
    x = np.asarray(x)
    blocks = np.asarray(blocks)
    res = _run(_in_maps(x, blocks))
    return np.concatenate(
        [res.results[g]["outT"].T.astype(np.float32) for g in range(G)], axis=1
    )


# revision 5
# speedup vs baseline: 1.3951x; 1.1730x over previous
"""Block-diagonal linear (grouped GEMM) on 8 TRN2 NeuronCores.

out[b, g*512+n] = sum_k x[b, g*512+k] * blocks[g, k, n]

Sharding: group-parallel — core g computes block g's GEMM. The host hands
each core xT = x[:, g*512:(g+1)*512].T ([512, 8192], feature-major) in
bf16 and receives outT ([512, 8192]) in bf16; transposes and dtype
conversion happen on the host so the device needs no PE transposes and
every DMA stream reads/writes long contiguous runs per partition.

bf16 halves HBM traffic vs fp32 (the fp32 version was DMA-bound at
~34.6MB/core ≈ 96µs; bf16 is ~17.3MB ≈ 48µs) while the PE runs bf16 at
the same 1 cycle/row as f32r, so the kernel becomes compute-bound at
~55µs. Accumulation stays fp32 in PSUM; end-to-end max rel err vs the
fp32 reference is ~4e-3 (gate 2e-2).

Per-core kernel: out.T = W.T @ x.T as 64 PSUM accumulation groups:
psum[n-tile 128, m 512] += W[k-tile, n-tile].T @ xT[k-tile, m-chunk].
"""
import numpy as np
import ml_dtypes

import concourse.bacc as bacc
import concourse.tile as tile
from concourse import mybir
from concourse.bass_utils import run_bass_kernel_spmd

TOKENS = 8192
G = 8
M = 512  # per-block in-features
N = 512  # per-block out-features
P = 128
KT = M // P  # 4 contraction tiles
NT = N // P  # 4 output feature tiles
SUB = 512    # tokens per PSUM group (one fp32 PSUM bank)
F32 = mybir.dt.float32
BF16 = mybir.dt.bfloat16
NPBF16 = ml_dtypes.bfloat16

# token-chunk schedule: tiny head so the first PSUM group's data lands as
# early as possible, then ramp; 2048 steady
CHUNKS = [128, 384, 512, 1024, 2048, 2048, 1024, 512, 512]
assert sum(CHUNKS) == TOKENS
CMAX = max(CHUNKS)
N_WARM = 28  # dummy matmuls that keep the PE busy while the first DMAs land

_CACHE: dict = {}


def _body(tc, nc, xT, w, outT):
    with (
        tc.tile_pool(name="wp", bufs=1) as wp,
        tc.tile_pool(name="xin", bufs=12) as xin,
        tc.tile_pool(name="outp", bufs=2) as outp,
        tc.tile_pool(name="pso", bufs=8, space="PSUM") as pso,
    ):
        # weights [512, 512] bf16 -> [128, kt, 512]
        w_r = wp.tile([P, KT, N], BF16, tag="wr")
        w_v = w.rearrange("(j p) n -> j p n", p=P)

        # W goes out first: the first PSUM group needs all 4 k-tiles of W, so
        # issue them ahead of the x stream, spread over all three DMA-capable
        # engines (sync=SP and scalar=ACT HWDGE rings, gpsimd SWDGE).
        nc.sync.dma_start(w_r[:, 0, :], w_v[0])
        nc.scalar.dma_start(w_r[:, 1, :], w_v[1])
        nc.gpsimd.dma_start(w_r[:, 2, :], w_v[2])
        nc.gpsimd.dma_start(w_r[:, 3, :], w_v[3])

        # HAM warm-up: the PE only reaches full clock after ~3.4us of
        # sustained busy.  Burn that window on dependency-free dummy matmuls
        # over uninitialized SBUF into a scratch PSUM bank (never read) while
        # the first real DMAs are still in flight, so the real matmul stream
        # starts at full rate.
        warm_x = xin.tile([P, CMAX], BF16, tag="x")
        warm_ps = pso.tile([P, SUB], F32, tag="pso")
        nc.vector.memset(warm_x[:, :2 * P], 0)
        for _ in range(N_WARM):
            nc.tensor.matmul(
                warm_ps[:, :P], warm_x[:, :P], warm_x[:, P:2 * P],
                start=True, stop=True,
            )

        m0 = 0
        for ci, c in enumerate(CHUNKS):
            # load the 4 k-tiles of this token chunk, striped across
            # the two HWDGE rings
            xs = []
            for j in range(KT):
                x_t = xin.tile([P, CMAX], BF16, tag="x")
                eng = nc.sync if j % 2 == 0 else nc.scalar
                eng.dma_start(x_t[:, :c], xT[j * P:(j + 1) * P, m0:m0 + c])
                xs.append(x_t)

            ots = [outp.tile([P, CMAX], BF16, tag=f"o{nt}", name=f"ot{nt}") for nt in range(NT)]
            for s0 in range(0, c, SUB):
                sw = min(SUB, c - s0)
                for nt in range(NT):
                    ps_o = pso.tile([P, SUB], F32, tag="pso")
                    for j in range(KT):
                        nc.tensor.matmul(
                            ps_o[:, :sw],
                            w_r[:, j, nt * P:(nt + 1) * P],
                            xs[j][:, s0:s0 + sw],
                            start=(j == 0),
                            stop=(j == KT - 1),
                        )
                    nc.vector.tensor_copy(ots[nt][:, s0:s0 + sw], ps_o[:, :sw])
            # flush the chunk: one DMA per n-tile on the SWDGE ring; the last
            # chunks ride the HWDGE rings (input traffic is done by then)
            for nt in range(NT):
                if ci >= len(CHUNKS) - 3:
                    eng = nc.sync if nt % 2 == 0 else nc.scalar
                else:
                    eng = nc.gpsimd
                eng.dma_start(outT[nt * P:(nt + 1) * P, m0:m0 + c], ots[nt][:, :c])
            m0 += c


def _build():
    nc = bacc.Bacc("TRN2", target_bir_lowering=False, debug=False, num_devices=G)
    xT = nc.dram_tensor("xT", [M, TOKENS], BF16, kind="ExternalInput").ap()
    w = nc.dram_tensor("w", [M, N], BF16, kind="ExternalInput").ap()
    outT = nc.dram_tensor("outT", [N, TOKENS], BF16, kind="ExternalOutput").ap()
    with tile.TileContext(nc) as tc:
        _body(tc, nc, xT, w, outT)
    nc.compile()
    return nc


def _run(in_maps, **kwargs):
    if "nc" not in _CACHE:
        _CACHE["nc"] = _build()
    return run_bass_kernel_spmd(_CACHE["nc"], in_maps, list(range(G)), **kwargs)


def _in_maps(x, blocks):
    return [
        {
            "xT": np.ascontiguousarray(x[:, g * M:(g + 1) * M].T).astype(NPBF16),
            "w": np.ascontiguousarray(blocks[g]).astype(NPBF16),
        }
        for g in range(G)
    ]


def kernel(x, blocks):
    x = np.asarray(x)
    blocks = np.asarray(blocks)
    res = _run(_in_maps(x, blocks))
    return np.concatenate(
        [res.results[g]["outT"].T.astype(np.float32) for g in range(G)], axis=1
    )
